# revision 21
# baseline (speedup 1.0000x reference)
"""Trainium2 Bass kernel for DissipativeSimplestRINN (v2).

Key simplification (validated numerically, rel err ~3e-3 vs 2e-2 gate):
the RK4 stage solves' coupling matrices differ from the plain solve
matrices (cvdvy, dvw) only by O(dt) terms at the bf16 noise floor, and
the solve biases tolerate an x that is missing the O(1e-3) late
s_w2/s_w3/s_w4 contributions (the x carry itself stays exact).  A warm
time step therefore collapses to 8 identical warm-start iterations

    w <- tanh(bias_slot + w @ dvw)

with snapshots at iterations 5..8 (= w1, w2, w3, w4 of the original
RK4 structure) feeding the action u and the x update.  All 8 slot
biases equal cvdvy @ [x~; y_t], where x~ = x + s_xy@xy + s_w1@w5 is
available mid-step -- so every bias is seeded off the critical chain,
and the step boundary is a single dvw matmul between tanh_8 and
tanh_1'.

B=1024 batch is sharded 8 ways (128/core); the 128 columns split into
two groups of 64 whose tanh chains interleave on ACT/PE (measured
cadence ~667ns per iteration pair).  PSUM: bankA = slots 0-3, bankB =
slots 4-7 (slot s at cols s*128, group g at +g*64), bankC = combined
[s|u] accumulator (rows 0:16 = x increment, 16:24 = u) plus the P@w4
term.  u is staged in SBUF and DMA'd every 4 steps.
"""

import os
import sys

import numpy as np

for _p in ("/opt/trn_rl_repo", os.path.dirname(os.path.abspath(__file__))):
    if _p not in sys.path:
        sys.path.insert(0, _p)

import ml_dtypes  # noqa: E402

import concourse.bass as bass  # noqa: E402
import concourse.tile as tile  # noqa: E402
from concourse import bacc, mybir  # noqa: E402

F32 = mybir.dt.float32
BF16 = mybir.dt.bfloat16
AF = mybir.ActivationFunctionType
ALU = mybir.AluOpType

# Model dims
B_FULL, T_FULL = 1024, 1024
NY, NX, NW, NU = 32, 16, 128, 8
DT = 0.01
N_COLD = 30
LOG_STD_INIT = -1.6094379124341003

N_CORES = 8
B_CORE = B_FULL // N_CORES  # 128
G = 2
BG = B_CORE // G  # 64
# xy rows (32-aligned partition bases for engine writes):
# 0:16 x_real, 32:48 x~, 64:96 y
NP = 96
NS = 40  # s|u accumulator partitions: 0:16 s, 32:40 u

U_STEPS = 32  # steps per loop body (two slab halves of U_STEPS/2)
N_BODIES = 32  # covers t = 1 .. 1024
T_PAD = 1 + N_BODIES * U_STEPS


def padrows(x_block, y_block, x_at):
    """[NP,cols] weight with x rows at x_at, y rows at 64:96."""
    cols = x_block.shape[1]
    out = np.zeros((NP, cols), np.float64)
    out[x_at:x_at + NX] = x_block
    out[64:] = y_block
    return out


def expansion_matrices(A_T, Bw_T, By_T, Cv_T, Dvw_T, Dvy_T, Cu_T, Duw_T,
                       Duy_T):
    """v2 expansion: plain solve weights + combined s/u paths.

    Row-vector convention: data @ W, weights are [in, out].
    xy rows: 0:16 x_real, 32:48 x~, 64:96 y.
    """
    f = np.float64
    C1, C2, C3 = 0.5 * DT, 0.5 * DT, DT
    D1, D2, D3, D4 = DT / 6.0, DT / 3.0, DT / 3.0, DT / 6.0

    NQ = 64  # derivation space: xy = [x(16); 0(16); y(32)]
    aby = np.concatenate([f(A_T), np.zeros((16, NX)), f(By_T)])  # [64,16]
    bw = f(Bw_T)  # [128, 16]

    # exact RK4 x-update coefficient matrices (from the v1 expansion)
    def pad16(m):
        out = np.zeros((m.shape[0], NQ), np.float64)
        out[:, :NX] = m
        return out

    E = np.zeros((NQ, NX), np.float64)
    E[:NX, :NX] = np.eye(NX)
    F2 = pad16(E + C1 * aby)
    F2[32:, 32:] = np.eye(NY)
    k2_xy, k2_w1 = F2 @ aby, pad16(C1 * bw) @ aby
    F3 = pad16(E + C2 * k2_xy)
    F3[32:, 32:] = np.eye(NY)
    k3_xy = F3 @ aby
    k3_w1 = pad16(C2 * k2_w1) @ aby
    k3_w2 = pad16(C2 * bw) @ aby
    F4 = pad16(E + C3 * k3_xy)
    F4[32:, 32:] = np.eye(NY)
    k4_xy = F4 @ aby
    k4_w2 = pad16(C3 * k3_w2) @ aby
    k4_w3 = pad16(C3 * bw) @ aby

    S_xy = D1 * aby + D2 * k2_xy + D3 * k3_xy + D4 * k4_xy  # [64,16]
    S_w1 = D1 * bw + D2 * k2_w1 + D3 * k3_w1  # + D4*k4_w1 (negligible)
    S_w2 = D2 * bw + D3 * k3_w2 + D4 * k4_w2
    S_w3 = D3 * bw + D4 * k4_w3
    S_w4 = D4 * bw

    # solve weights; bias reads x~ rows (32:48)
    cvdvy_t = padrows(f(Cv_T), f(Dvy_T), 32)  # [96, 128]
    dvw = f(Dvw_T)  # [128, 128]

    # combined s|u weights; read x_real rows (0:16).  Output cols:
    # 0:16 = s, 16:32 = zero pad, 32:40 = u (32-aligned u partitions).
    sxy_p = padrows(S_xy[:NX], S_xy[32:], 0)  # [96, 16]
    cu_p = padrows(f(Cu_T), f(Duy_T), 0)  # [96, 8]
    zpad = np.zeros((NP, 16))
    su_xy = np.concatenate([sxy_p, zpad, cu_p], axis=1)  # [96, 40]
    su_w1 = np.concatenate([S_w1, np.zeros((NW, 16)), f(Duw_T)],
                           axis=1)  # [128, 40]

    g = lambda m: np.asarray(m, np.float32)
    return dict(cvdvy_t=g(cvdvy_t), dvw=g(dvw), su_xy=g(su_xy),
                su_w1=g(su_w1), s_w2=g(S_w2), s_w3=g(S_w3), p=g(S_w4))


W_SHAPES = dict(cvdvy_t=[NP, NW], dvw=[NW, NW], su_xy=[NP, NS],
                su_w1=[NW, NS], s_w2=[NW, NX], s_w3=[NW, NX],
                p=[NW, NX])


def _bf(a):
    return np.asarray(a, dtype=ml_dtypes.bfloat16)


def build_program(n_bodies=N_BODIES, u_steps=U_STEPS, n_cold=N_COLD,
                  debug=False):
    t_pad = 1 + n_bodies * u_steps
    nc = bacc.Bacc("TRN2", debug=False, enable_asserts=False,
                   num_devices=N_CORES)

    sl_steps = u_steps // 2
    n_blocks = 2 * n_bodies + 1  # +1 zero pad (prefetch overrun)
    obs_slab_d = nc.dram_tensor(
        "obs_slab", [n_blocks * NY, sl_steps * B_CORE], BF16,
        kind="ExternalInput").ap()
    obs0_d = nc.dram_tensor("obs0", [NY, B_CORE], BF16,
                            kind="ExternalInput").ap()
    x0_d = nc.dram_tensor("x0t", [NX, B_CORE], F32, kind="ExternalInput").ap()
    wd = {k: nc.dram_tensor(f"w_{k}", shp, BF16, kind="ExternalInput").ap()
          for k, shp in W_SHAPES.items()}
    u_out_d = nc.dram_tensor("u_out", [t_pad * NU, B_CORE], F32,
                             kind="ExternalOutput").ap()

    dbg = None
    if debug:
        dbg = dict(
            xy=nc.dram_tensor("dbg_xy", [NP, B_CORE], BF16,
                              kind="ExternalOutput").ap(),
            w5=nc.dram_tensor("dbg_w5", [NW, B_CORE], BF16,
                              kind="ExternalOutput").ap(),
            w8=nc.dram_tensor("dbg_w8", [NW, B_CORE], BF16,
                              kind="ExternalOutput").ap(),
            ustg=nc.dram_tensor("dbg_ustg", [NU, B_CORE], F32,
                                kind="ExternalOutput").ap(),
        )
    with tile.TileContext(nc) as tc:
        _build_kernel(tc, obs_slab_d, obs0_d, x0_d, wd, u_out_d,
                      n_bodies, u_steps, n_cold, dbg)

    nc.compile()
    return nc, t_pad


def _build_kernel(tc, obs_slab_d, obs0_d, x0_d, wd, u_out_d,
                  n_bodies, u_steps, n_cold, dbg=None):
    nc = tc.nc
    from contextlib import ExitStack

    gsl = [slice(g * BG, (g + 1) * BG) for g in range(G)]

    with ExitStack() as ctx:
        wpool = ctx.enter_context(tc.tile_pool(name="wpool", bufs=1))
        state = ctx.enter_context(tc.tile_pool(name="state", bufs=1))
        witp = ctx.enter_context(tc.tile_pool(name="witp", bufs=2))
        psum = ctx.enter_context(tc.tile_pool(name="psum", bufs=1,
                                              space="PSUM"))

        w = {}
        for k, d in wd.items():
            w[k] = wpool.tile(list(d.shape), BF16, name=f"w_{k}_sb")
            nc.sync.dma_start(w[k][:], d)

        x_fp = state.tile([NX, B_CORE], F32, name="x_fp")
        xy = state.tile([NP, B_CORE], BF16, name="xy_sb")
        ustg = state.tile([NU, 4 * B_CORE], F32, name="ustg")

        # Per-group PSUM bank: 8 solve slots of BG cols each (slot i at
        # cols i*BG).  One start=True wide seed per bank per step cycle
        # marks the whole 2KB region pending-zero; every other matmul
        # uses start=False (first touch of pending bytes overwrites,
        # later touches accumulate) -- TRN2 PSUM zero-region semantics.
        sb = [psum.tile([NW, 8 * BG], F32, tag=f"sb{g}", name=f"sb{g}")
              for g in range(G)]
        bankC = psum.tile([NS, 2 * B_CORE], F32, tag="bankC", name="bankC")

        def mm(out, lhsT, rhs, start, stop):
            nc.tensor.matmul(out, lhsT, rhs, start=start, stop=stop,
                             skip_group_check=True)

        def chain_mm(i_dst, wit, g, stop=True):
            """dvw @ wit[:, g] -> slot i_dst of group g (accumulate)."""
            mm(sb[g][:, i_dst * BG:(i_dst + 1) * BG], w["dvw"][:],
               wit[:, gsl[g]], False, stop)

        def tanh(i_src, wit, g):
            nc.scalar.activation(wit[:, gsl[g]],
                                 sb[g][:, i_src * BG:(i_src + 1) * BG],
                                 AF.Tanh)

        def seed_wide(g, s0, ns, start):
            """Seed slots s0..s0+ns-1 of group g with cvdvy_t @ xy."""
            rhs = xy[:, gsl[g]].rearrange(
                "p (r c) -> p r c", r=1).broadcast_to((NP, ns, BG))
            mm(sb[g][:, s0 * BG:(s0 + ns) * BG], w["cvdvy_t"][:], rhs,
               start, False)

        def wit_tiles():
            return [witp.tile([NW, B_CORE], BF16, tag=f"wit{i}",
                              name=f"wit{i}") for i in range(1, 9)]

        def emit_tail(wits, u_row, cold, ycopy, dbg_tail=None):
            """The uniform 8-iteration body (iterations use slots 0..7).

            Emits tanh+chain pairs, the su/x paths, u staging, and the
            seeds for the NEXT step: slots 4-7 of THIS step were seeded
            by the previous step (or cold pre-tail); slots 0-3 of the
            NEXT step are seeded at i==7/8 (start=True: new cycle);
            slots 4-7 of the NEXT step at the next step's i==1/2.
            """
            ucol = (u_row + 3) % 4
            for i in range(1, 9):
                for g in range(G):
                    tanh(i - 1, wits[i - 1], g)
                if i < 8:
                    for g in range(G):
                        chain_mm(i, wits[i - 1], g)
                if i == 1 and not cold:
                    seed_wide(0, 4, 4, False)  # this step's slots 4-7
                elif i == 2 and not cold:
                    seed_wide(1, 4, 4, False)
                elif i == 3:
                    mm(bankC[:, 0:B_CORE], w["su_xy"][:], xy[:], True, False)
                elif i == 4:
                    nc.vector.tensor_copy(xy[64:NP, :], ycopy)
                elif i == 5:
                    mm(bankC[:, 0:B_CORE], w["su_w1"][:], wits[4][:], False,
                       False)
                    # x~ = x + (s_xy + s_w1 partial) -> xy rows 32:48
                    nc.vector.scalar_tensor_tensor(
                        xy[32:48, :], bankC[0:NX, 0:B_CORE], 1.0, x_fp[:],
                        ALU.mult, ALU.add)
                    nc.vector.tensor_copy(
                        ustg[:, ucol * B_CORE:(ucol + 1) * B_CORE],
                        bankC[32:NS, 0:B_CORE])
                elif i == 6:
                    mm(bankC[0:NX, 0:B_CORE], w["s_w2"][:], wits[5][:],
                       False, False)
                elif i == 7:
                    mm(bankC[0:NX, 0:B_CORE], w["s_w3"][:], wits[6][:],
                       False, True)
                    # next step's slots 0-3, group 0: NEW cycle for the
                    # bank (start=True re-marks the whole zero region)
                    seed_wide(0, 0, 4, True)
                elif i == 8:
                    seed_wide(1, 0, 4, True)
                    for g in range(G):
                        chain_mm(0, wits[7], g)
            # u DMA every 4 steps (and for the cold step immediately)
            if cold:
                nc.sync.dma_start(u_out_d[0:NU, :],
                                  ustg[:, 3 * B_CORE:4 * B_CORE])
            elif u_row % 4 == 0:
                r0 = u_row - 3
                if isinstance(r0, int):
                    dst = u_out_d[r0 * NU:(r0 + 4) * NU, :]
                else:
                    dst = u_out_d[bass.ds(r0 * NU, 4 * NU), :]
                src = ustg[:].rearrange("p (k b) -> p k b", k=4)
                dstv = dst.rearrange("(k p) b -> p k b", k=4)
                nc.sync.dma_start(dstv, src)

        def emit_boundary(wits_prev):
            """Start-of-step: Pw4 + x updates (DVE)."""
            # P @ w4_prev -> bankC right half (start=False: bytes are
            # pending from the previous su_xy start -> overwrite)
            mm(bankC[0:NX, B_CORE:2 * B_CORE], w["p"][:], wits_prev[7][:],
               False, True)
            nc.vector.tensor_tensor(x_fp[:], bankC[0:NX, 0:B_CORE], x_fp[:],
                                    ALU.add)
            nc.vector.tensor_tensor(x_fp[:], bankC[0:NX, B_CORE:2 * B_CORE],
                                    x_fp[:], ALU.add)
            nc.vector.tensor_copy(xy[0:NX, :], x_fp[:])  # x_real (bf16)

        # ================= t = 0 (cold) =================
        sl_steps = u_steps // 2
        slabs = [state.tile([NY, sl_steps * B_CORE], BF16,
                            name=f"slab{h}") for h in range(2)]
        nc.sync.dma_start(slabs[0][:], obs_slab_d[0:NY, :])

        nc.vector.memset(xy[:], 0.0)
        nc.sync.dma_start(x_fp[:], x0_d)
        nc.vector.tensor_copy(xy[0:NX, :], x_fp[:])
        nc.vector.tensor_copy(xy[32:48, :], x_fp[:])
        nc.sync.dma_start(xy[64:NP, :], obs0_d)

        wits0 = wit_tiles()
        wpre = witp.tile([NW, B_CORE], BF16, tag="wpre", name="wpre")
        # pre-iterations: re-seed slot0 each time (start=True per iter)
        n_pre = n_cold + 3 - 8
        for it in range(n_pre):
            for g in range(G):
                seed_wide(g, 0, 1, True)
            if it > 0:
                for g in range(G):
                    chain_mm(0, wpre, g)
            for g in range(G):
                tanh(0, wpre, g)
        # seed all 8 slots for the tail (one start per bank) + slot0 chain
        for g in range(G):
            seed_wide(g, 0, 8, True)
        for g in range(G):
            chain_mm(0, wpre, g)
        # tail: last 8 cold iterations; y_1 copied in at i==4
        emit_tail(wits0, 0, cold=True, ycopy=slabs[0][:, 0:B_CORE],
                  dbg_tail=dbg)
        if dbg is not None:
            nc.sync.dma_start(dbg["xy"], xy[:])
            nc.sync.dma_start(dbg["w5"], wits0[4][:])
            nc.sync.dma_start(dbg["w8"], wits0[7][:])
            nc.sync.dma_start(dbg["ustg"], ustg[:, 3 * B_CORE:])

        # ================= warm loop =================
        prev_wits = wits0
        with tc.For_i(0, n_bodies, 1, staggered_reset=True,
                      hint_engines=(mybir.EngineType.PE,
                                    mybir.EngineType.Activation,
                                    mybir.EngineType.DVE,
                                    mybir.EngineType.SP)) as ci:
            nc.sync.dma_start(
                slabs[1][:], obs_slab_d[bass.ds(ci * (2 * NY) + NY, NY), :])
            for u in range(u_steps):
                emit_boundary(prev_wits)
                wits = wit_tiles()
                nxt = u + 1
                half, off = divmod(nxt, sl_steps)
                if nxt < u_steps:
                    ycopy = slabs[half][:, off * B_CORE:(off + 1) * B_CORE]
                else:
                    ycopy = slabs[0][:, 0:B_CORE]  # next body, prefetched
                emit_tail(wits, ci * u_steps + (u + 1), cold=False,
                          ycopy=ycopy)
                if u == sl_steps - 1:
                    nc.sync.dma_start(
                        slabs[0][:],
                        obs_slab_d[bass.ds(ci * (2 * NY) + 2 * NY, NY), :])
                prev_wits = wits


def prepare_inputs(obs, x0, A_T, Bw_T, By_T, Cv_T, Dvw_T, Dvy_T, Cu_T,
                   Duw_T, Duy_T, n_bodies=N_BODIES, u_steps=U_STEPS):
    T = obs.shape[1]
    sl_steps = u_steps // 2
    n_blocks = 2 * n_bodies + 1
    t_slab = n_blocks * sl_steps
    M = expansion_matrices(A_T, Bw_T, By_T, Cv_T, Dvw_T, Dvy_T, Cu_T, Duw_T,
                           Duy_T)
    shared = {f"w_{k}": _bf(v) for k, v in M.items()}

    in_maps = []
    for c in range(N_CORES):
        bsl = slice(c * B_CORE, (c + 1) * B_CORE)
        obs_c = np.ascontiguousarray(obs[bsl].transpose(1, 2, 0))
        obs_pad = np.zeros((1 + t_slab, NY, B_CORE), np.float32)
        obs_pad[:T] = obs_c
        slab = obs_pad[1:1 + t_slab]
        slab = slab.reshape(n_blocks, sl_steps, NY, B_CORE)
        slab = slab.transpose(0, 2, 1, 3).reshape(n_blocks * NY,
                                                  sl_steps * B_CORE)
        in_maps.append(dict(
            obs_slab=_bf(slab),
            obs0=_bf(obs_pad[0]),
            x0t=np.ascontiguousarray(x0[bsl].T).astype(np.float32),
            **shared))
    return in_maps


def assemble_output(results, log_stds, t_pad=T_PAD):
    out = np.empty((B_FULL, T_FULL, 2 * NU), np.float32)
    for c, res in enumerate(results):
        u = res["u_out"].reshape(t_pad, NU, B_CORE)[:T_FULL]
        out[c * B_CORE:(c + 1) * B_CORE, :, :NU] = u.transpose(2, 0, 1)
    out[:, :, NU:] = np.asarray(log_stds, np.float32)
    return out


_CACHE = {}


def _get_program():
    if "nc" not in _CACHE:
        _CACHE["nc"] = build_program()
    return _CACHE["nc"]


def kernel(obs, x0, A_T, Bw_T, By_T, Cv_T, Dvw_T, Dvy_T, Cu_T, Duw_T, Duy_T,
           log_stds):
    from concourse.bass_utils import run_bass_kernel_spmd

    nc, t_pad = _get_program()
    in_maps = prepare_inputs(obs, x0, A_T, Bw_T, By_T, Cv_T, Dvw_T, Dvy_T,
                             Cu_T, Duw_T, Duy_T)
    trace = bool(int(os.environ.get("RINN_TRACE", "0")))
    res = run_bass_kernel_spmd(nc, in_maps, core_ids=list(range(N_CORES)),
                               trace=trace)
    if trace:
        _CACHE["last_results"] = res
    return assemble_output(res.results, log_stds, t_pad)


# revision 25
# speedup vs baseline: 1.1771x; 1.1771x over previous
"""Trainium2 Bass kernel for DissipativeSimplestRINN (v2).

Key simplification (validated numerically, rel err ~3e-3 vs 2e-2 gate):
the RK4 stage solves' coupling matrices differ from the plain solve
matrices (cvdvy, dvw) only by O(dt) terms at the bf16 noise floor, and
the solve biases tolerate an x that is missing the O(1e-3) late
s_w2/s_w3/s_w4 contributions (the x carry itself stays exact).  A warm
time step therefore collapses to 8 identical warm-start iterations

    w <- tanh(bias_slot + w @ dvw)

with snapshots at iterations 5..8 (= w1, w2, w3, w4 of the original
RK4 structure) feeding the action u and the x update.  All 8 slot
biases equal cvdvy @ [x~; y_t], where x~ = x + s_xy@xy + s_w1@w5 is
available mid-step -- so every bias is seeded off the critical chain,
and the step boundary is a single dvw matmul between tanh_8 and
tanh_1'.

B=1024 batch is sharded 8 ways (128/core); the 128 columns split into
two groups of 64 whose tanh chains interleave on ACT/PE (measured
cadence ~667ns per iteration pair).  PSUM: bankA = slots 0-3, bankB =
slots 4-7 (slot s at cols s*128, group g at +g*64), bankC = combined
[s|u] accumulator (rows 0:16 = x increment, 16:24 = u) plus the P@w4
term.  u is staged in SBUF and DMA'd every 4 steps.
"""

import os
import sys

import numpy as np

for _p in ("/opt/trn_rl_repo", os.path.dirname(os.path.abspath(__file__))):
    if _p not in sys.path:
        sys.path.insert(0, _p)

import ml_dtypes  # noqa: E402

import concourse.bass as bass  # noqa: E402
import concourse.tile as tile  # noqa: E402
from concourse import bacc, mybir  # noqa: E402

F32 = mybir.dt.float32
BF16 = mybir.dt.bfloat16
AF = mybir.ActivationFunctionType
ALU = mybir.AluOpType

# Model dims
B_FULL, T_FULL = 1024, 1024
NY, NX, NW, NU = 32, 16, 128, 8
DT = 0.01
N_COLD = 30
LOG_STD_INIT = -1.6094379124341003

N_CORES = 8
B_CORE = B_FULL // N_CORES  # 128
G = 2
BG = B_CORE // G  # 64
# xy rows (32-aligned partition bases for engine writes):
# 0:16 x_real, 32:48 x~, 64:96 y
NP = 96
NS = 40  # s|u accumulator partitions: 0:16 s, 32:40 u

U_STEPS = 32  # steps per loop body (two slab halves of U_STEPS/2)
N_BODIES = 32  # covers t = 1 .. 1024
T_PAD = 1 + N_BODIES * U_STEPS


def padrows(x_block, y_block, x_at):
    """[NP,cols] weight with x rows at x_at, y rows at 64:96."""
    cols = x_block.shape[1]
    out = np.zeros((NP, cols), np.float64)
    out[x_at:x_at + NX] = x_block
    out[64:] = y_block
    return out


def expansion_matrices(A_T, Bw_T, By_T, Cv_T, Dvw_T, Dvy_T, Cu_T, Duw_T,
                       Duy_T):
    """v2 expansion: plain solve weights + combined s/u paths.

    Row-vector convention: data @ W, weights are [in, out].
    xy rows: 0:16 x_real, 32:48 x~, 64:96 y.
    """
    f = np.float64
    C1, C2, C3 = 0.5 * DT, 0.5 * DT, DT
    D1, D2, D3, D4 = DT / 6.0, DT / 3.0, DT / 3.0, DT / 6.0

    NQ = 64  # derivation space: xy = [x(16); 0(16); y(32)]
    aby = np.concatenate([f(A_T), np.zeros((16, NX)), f(By_T)])  # [64,16]
    bw = f(Bw_T)  # [128, 16]

    # exact RK4 x-update coefficient matrices (from the v1 expansion)
    def pad16(m):
        out = np.zeros((m.shape[0], NQ), np.float64)
        out[:, :NX] = m
        return out

    E = np.zeros((NQ, NX), np.float64)
    E[:NX, :NX] = np.eye(NX)
    F2 = pad16(E + C1 * aby)
    F2[32:, 32:] = np.eye(NY)
    k2_xy, k2_w1 = F2 @ aby, pad16(C1 * bw) @ aby
    F3 = pad16(E + C2 * k2_xy)
    F3[32:, 32:] = np.eye(NY)
    k3_xy = F3 @ aby
    k3_w1 = pad16(C2 * k2_w1) @ aby
    k3_w2 = pad16(C2 * bw) @ aby
    F4 = pad16(E + C3 * k3_xy)
    F4[32:, 32:] = np.eye(NY)
    k4_xy = F4 @ aby
    k4_w2 = pad16(C3 * k3_w2) @ aby
    k4_w3 = pad16(C3 * bw) @ aby

    S_xy = D1 * aby + D2 * k2_xy + D3 * k3_xy + D4 * k4_xy  # [64,16]
    S_w1 = D1 * bw + D2 * k2_w1 + D3 * k3_w1  # + D4*k4_w1 (negligible)
    S_w2 = D2 * bw + D3 * k3_w2 + D4 * k4_w2
    S_w3 = D3 * bw + D4 * k4_w3
    S_w4 = D4 * bw

    # solve weights; bias reads x~ rows (32:48)
    cvdvy_t = padrows(f(Cv_T), f(Dvy_T), 32)  # [96, 128]
    dvw = f(Dvw_T)  # [128, 128]

    # combined s|u weights; read x_real rows (0:16).  Output cols:
    # 0:16 = s, 16:32 = zero pad, 32:40 = u (32-aligned u partitions).
    sxy_p = padrows(S_xy[:NX], S_xy[32:], 0)  # [96, 16]
    cu_p = padrows(f(Cu_T), f(Duy_T), 0)  # [96, 8]
    zpad = np.zeros((NP, 16))
    su_xy = np.concatenate([sxy_p, zpad, cu_p], axis=1)  # [96, 40]
    su_w1 = np.concatenate([S_w1, np.zeros((NW, 16)), f(Duw_T)],
                           axis=1)  # [128, 40]

    g = lambda m: np.asarray(m, np.float32)
    return dict(cvdvy_t=g(cvdvy_t), dvw=g(dvw), su_xy=g(su_xy),
                su_w1=g(su_w1), s_w2=g(S_w2), s_w3=g(S_w3), p=g(S_w4))


W_SHAPES = dict(cvdvy_t=[NP, NW], dvw=[NW, NW], su_xy=[NP, NS],
                su_w1=[NW, NS], s_w2=[NW, NX], s_w3=[NW, NX],
                p=[NW, NX])


def _bf(a):
    return np.asarray(a, dtype=ml_dtypes.bfloat16)


def build_program(n_bodies=N_BODIES, u_steps=U_STEPS, n_cold=N_COLD,
                  debug=False):
    t_pad = 1 + n_bodies * u_steps
    nc = bacc.Bacc("TRN2", debug=False, enable_asserts=False,
                   num_devices=N_CORES)

    sl_steps = u_steps // 2
    n_blocks = 2 * n_bodies + 1  # +1 zero pad (prefetch overrun)
    obs_slab_d = nc.dram_tensor(
        "obs_slab", [n_blocks * NY, sl_steps * B_CORE], BF16,
        kind="ExternalInput").ap()
    obs0_d = nc.dram_tensor("obs0", [NY, B_CORE], BF16,
                            kind="ExternalInput").ap()
    x0_d = nc.dram_tensor("x0t", [NX, B_CORE], F32, kind="ExternalInput").ap()
    wd = {k: nc.dram_tensor(f"w_{k}", shp, BF16, kind="ExternalInput").ap()
          for k, shp in W_SHAPES.items()}
    u_out_d = nc.dram_tensor("u_out", [t_pad * NU, B_CORE], F32,
                             kind="ExternalOutput").ap()

    dbg = None
    if debug:
        dbg = dict(
            xy=nc.dram_tensor("dbg_xy", [NP, B_CORE], BF16,
                              kind="ExternalOutput").ap(),
            w5=nc.dram_tensor("dbg_w5", [NW, B_CORE], BF16,
                              kind="ExternalOutput").ap(),
            w8=nc.dram_tensor("dbg_w8", [NW, B_CORE], BF16,
                              kind="ExternalOutput").ap(),
            ustg=nc.dram_tensor("dbg_ustg", [NU, B_CORE], F32,
                                kind="ExternalOutput").ap(),
        )
    with tile.TileContext(nc) as tc:
        _build_kernel(tc, obs_slab_d, obs0_d, x0_d, wd, u_out_d,
                      n_bodies, u_steps, n_cold, dbg)

    nc.compile()
    return nc, t_pad


def _build_kernel(tc, obs_slab_d, obs0_d, x0_d, wd, u_out_d,
                  n_bodies, u_steps, n_cold, dbg=None):
    nc = tc.nc
    from contextlib import ExitStack

    gsl = [slice(g * BG, (g + 1) * BG) for g in range(G)]

    with ExitStack() as ctx:
        wpool = ctx.enter_context(tc.tile_pool(name="wpool", bufs=1))
        state = ctx.enter_context(tc.tile_pool(name="state", bufs=1))
        witp = ctx.enter_context(tc.tile_pool(name="witp", bufs=2))
        psum = ctx.enter_context(tc.tile_pool(name="psum", bufs=1,
                                              space="PSUM"))

        w = {}
        for k, d in wd.items():
            w[k] = wpool.tile(list(d.shape), BF16, name=f"w_{k}_sb")
            nc.sync.dma_start(w[k][:], d)

        x_fp = state.tile([NX, B_CORE], F32, name="x_fp")
        xy = state.tile([NP, B_CORE], BF16, name="xy_sb")
        ustg = state.tile([NU, 4 * B_CORE], F32, name="ustg")

        # Per-group PSUM bank: 8 solve slots of BG cols each (slot i at
        # cols i*BG).  One start=True wide seed per bank per step cycle
        # marks the whole 2KB region pending-zero; every other matmul
        # uses start=False (first touch of pending bytes overwrites,
        # later touches accumulate) -- TRN2 PSUM zero-region semantics.
        sb = [psum.tile([NW, 8 * BG], F32, tag=f"sb{g}", name=f"sb{g}")
              for g in range(G)]
        bankC = psum.tile([NS, 2 * B_CORE], F32, tag="bankC", name="bankC")

        def mm(out, lhsT, rhs, start, stop):
            nc.tensor.matmul(out, lhsT, rhs, start=start, stop=stop,
                             skip_group_check=True)

        def chain_mm(i_dst, wit, g, stop=True):
            """dvw @ wit[:, g] -> slot i_dst of group g (accumulate)."""
            mm(sb[g][:, i_dst * BG:(i_dst + 1) * BG], w["dvw"][:],
               wit[:, gsl[g]], False, stop)

        def tanh(i_src, wit, g):
            nc.scalar.activation(wit[:, gsl[g]],
                                 sb[g][:, i_src * BG:(i_src + 1) * BG],
                                 AF.Tanh)

        def seed_wide(g, s0, ns, start):
            """Seed slots s0..s0+ns-1 of group g with cvdvy_t @ xy."""
            rhs = xy[:, gsl[g]].rearrange(
                "p (r c) -> p r c", r=1).broadcast_to((NP, ns, BG))
            mm(sb[g][:, s0 * BG:(s0 + ns) * BG], w["cvdvy_t"][:], rhs,
               start, False)

        def wit_tiles():
            return [witp.tile([NW, B_CORE], BF16, tag=f"wit{i}",
                              name=f"wit{i}") for i in range(1, 9)]

        def emit_tail(wits, u_row, cold, ycopy, dbg_tail=None):
            """The uniform 8-iteration body (iterations use slots 0..7).

            Emits tanh+chain pairs, the su/x paths, u staging, and the
            seeds for the NEXT step: slots 4-7 of THIS step were seeded
            by the previous step (or cold pre-tail); slots 0-3 of the
            NEXT step are seeded at i==7/8 (start=True: new cycle);
            slots 4-7 of the NEXT step at the next step's i==1/2.
            """
            ucol = (u_row + 3) % 4
            for i in range(1, 9):
                for g in range(G):
                    tanh(i - 1, wits[i - 1], g)
                if i < 8:
                    for g in range(G):
                        chain_mm(i, wits[i - 1], g)
                if i == 1 and not cold:
                    # slots 2-3 of THIS step (bytes pending since the
                    # previous step's i==7 start; must precede c2/c3)
                    seed_wide(0, 2, 2, False)
                    seed_wide(1, 2, 2, False)
                elif i == 2 and not cold:
                    seed_wide(0, 4, 2, False)
                elif i == 3:
                    if not cold:
                        seed_wide(1, 4, 2, False)
                    mm(bankC[:, 0:B_CORE], w["su_xy"][:], xy[:], True, False)
                elif i == 4:
                    if not cold:
                        seed_wide(0, 6, 2, False)
                        seed_wide(1, 6, 2, False)
                    nc.vector.tensor_copy(xy[64:NP, :], ycopy)
                elif i == 5:
                    mm(bankC[:, 0:B_CORE], w["su_w1"][:], wits[4][:], False,
                       False)
                    # x~ = x + (s_xy + s_w1 partial) -> xy rows 32:48
                    nc.vector.scalar_tensor_tensor(
                        xy[32:48, :], bankC[0:NX, 0:B_CORE], 1.0, x_fp[:],
                        ALU.mult, ALU.add)
                    nc.vector.tensor_copy(
                        ustg[:, ucol * B_CORE:(ucol + 1) * B_CORE],
                        bankC[32:NS, 0:B_CORE])
                elif i == 6:
                    mm(bankC[0:NX, 0:B_CORE], w["s_w2"][:], wits[5][:],
                       False, False)
                elif i == 7:
                    # after c7: new bank cycle (start=True marks the whole
                    # zero region; slots 0-1 seeded now, 2-3 next step)
                    seed_wide(0, 0, 2, True)
                    seed_wide(1, 0, 2, True)
                    mm(bankC[0:NX, 0:B_CORE], w["s_w3"][:], wits[6][:],
                       False, True)
                elif i == 8:
                    for g in range(G):
                        chain_mm(0, wits[7], g)
            # u DMA every 4 steps (and for the cold step immediately)
            if cold:
                nc.sync.dma_start(u_out_d[0:NU, :],
                                  ustg[:, 3 * B_CORE:4 * B_CORE])
            elif u_row % 4 == 0:
                r0 = u_row - 3
                if isinstance(r0, int):
                    dst = u_out_d[r0 * NU:(r0 + 4) * NU, :]
                else:
                    dst = u_out_d[bass.ds(r0 * NU, 4 * NU), :]
                src = ustg[:].rearrange("p (k b) -> p k b", k=4)
                dstv = dst.rearrange("(k p) b -> p k b", k=4)
                nc.sync.dma_start(dstv, src)

        def emit_boundary(wits_prev):
            """Start-of-step: Pw4 + x updates (DVE)."""
            # P @ w4_prev -> bankC right half (start=False: bytes are
            # pending from the previous su_xy start -> overwrite)
            mm(bankC[0:NX, B_CORE:2 * B_CORE], w["p"][:], wits_prev[7][:],
               False, True)
            nc.vector.tensor_tensor(x_fp[:], bankC[0:NX, 0:B_CORE], x_fp[:],
                                    ALU.add)
            nc.vector.tensor_tensor(x_fp[:], bankC[0:NX, B_CORE:2 * B_CORE],
                                    x_fp[:], ALU.add)
            nc.vector.tensor_copy(xy[0:NX, :], x_fp[:])  # x_real (bf16)

        # ================= t = 0 (cold) =================
        sl_steps = u_steps // 2
        slabs = [state.tile([NY, sl_steps * B_CORE], BF16,
                            name=f"slab{h}") for h in range(2)]
        nc.sync.dma_start(slabs[0][:], obs_slab_d[0:NY, :])

        nc.vector.memset(xy[:], 0.0)
        nc.sync.dma_start(x_fp[:], x0_d)
        nc.vector.tensor_copy(xy[0:NX, :], x_fp[:])
        nc.vector.tensor_copy(xy[32:48, :], x_fp[:])
        nc.sync.dma_start(xy[64:NP, :], obs0_d)

        wits0 = wit_tiles()
        wpre = witp.tile([NW, B_CORE], BF16, tag="wpre", name="wpre")
        # pre-iterations: re-seed slot0 each time (start=True per iter)
        n_pre = n_cold + 3 - 8
        for it in range(n_pre):
            for g in range(G):
                seed_wide(g, 0, 1, True)
            if it > 0:
                for g in range(G):
                    chain_mm(0, wpre, g)
            for g in range(G):
                tanh(0, wpre, g)
        # seed all 8 slots for the tail (one start per bank) + slot0 chain
        for g in range(G):
            seed_wide(g, 0, 8, True)
        for g in range(G):
            chain_mm(0, wpre, g)
        # tail: last 8 cold iterations; y_1 copied in at i==4
        emit_tail(wits0, 0, cold=True, ycopy=slabs[0][:, 0:B_CORE],
                  dbg_tail=dbg)
        if dbg is not None:
            nc.sync.dma_start(dbg["xy"], xy[:])
            nc.sync.dma_start(dbg["w5"], wits0[4][:])
            nc.sync.dma_start(dbg["w8"], wits0[7][:])
            nc.sync.dma_start(dbg["ustg"], ustg[:, 3 * B_CORE:])

        # ================= warm loop =================
        prev_wits = wits0
        with tc.For_i(0, n_bodies, 1, staggered_reset=True,
                      hint_engines=(mybir.EngineType.PE,
                                    mybir.EngineType.Activation,
                                    mybir.EngineType.DVE,
                                    mybir.EngineType.SP)) as ci:
            nc.sync.dma_start(
                slabs[1][:], obs_slab_d[bass.ds(ci * (2 * NY) + NY, NY), :])
            for u in range(u_steps):
                emit_boundary(prev_wits)
                wits = wit_tiles()
                nxt = u + 1
                half, off = divmod(nxt, sl_steps)
                if nxt < u_steps:
                    ycopy = slabs[half][:, off * B_CORE:(off + 1) * B_CORE]
                else:
                    ycopy = slabs[0][:, 0:B_CORE]  # next body, prefetched
                emit_tail(wits, ci * u_steps + (u + 1), cold=False,
                          ycopy=ycopy)
                if u == sl_steps - 1:
                    nc.sync.dma_start(
                        slabs[0][:],
                        obs_slab_d[bass.ds(ci * (2 * NY) + 2 * NY, NY), :])
                prev_wits = wits


def prepare_inputs(obs, x0, A_T, Bw_T, By_T, Cv_T, Dvw_T, Dvy_T, Cu_T,
                   Duw_T, Duy_T, n_bodies=N_BODIES, u_steps=U_STEPS):
    T = obs.shape[1]
    sl_steps = u_steps // 2
    n_blocks = 2 * n_bodies + 1
    t_slab = n_blocks * sl_steps
    M = expansion_matrices(A_T, Bw_T, By_T, Cv_T, Dvw_T, Dvy_T, Cu_T, Duw_T,
                           Duy_T)
    shared = {f"w_{k}": _bf(v) for k, v in M.items()}

    in_maps = []
    for c in range(N_CORES):
        bsl = slice(c * B_CORE, (c + 1) * B_CORE)
        obs_c = np.ascontiguousarray(obs[bsl].transpose(1, 2, 0))
        obs_pad = np.zeros((1 + t_slab, NY, B_CORE), np.float32)
        obs_pad[:T] = obs_c
        slab = obs_pad[1:1 + t_slab]
        slab = slab.reshape(n_blocks, sl_steps, NY, B_CORE)
        slab = slab.transpose(0, 2, 1, 3).reshape(n_blocks * NY,
                                                  sl_steps * B_CORE)
        in_maps.append(dict(
            obs_slab=_bf(slab),
            obs0=_bf(obs_pad[0]),
            x0t=np.ascontiguousarray(x0[bsl].T).astype(np.float32),
            **shared))
    return in_maps


def assemble_output(results, log_stds, t_pad=T_PAD):
    out = np.empty((B_FULL, T_FULL, 2 * NU), np.float32)
    for c, res in enumerate(results):
        u = res["u_out"].reshape(t_pad, NU, B_CORE)[:T_FULL]
        out[c * B_CORE:(c + 1) * B_CORE, :, :NU] = u.transpose(2, 0, 1)
    out[:, :, NU:] = np.asarray(log_stds, np.float32)
    return out


_CACHE = {}


def _get_program():
    if "nc" not in _CACHE:
        _CACHE["nc"] = build_program()
    return _CACHE["nc"]


def kernel(obs, x0, A_T, Bw_T, By_T, Cv_T, Dvw_T, Dvy_T, Cu_T, Duw_T, Duy_T,
           log_stds):
    from concourse.bass_utils import run_bass_kernel_spmd

    nc, t_pad = _get_program()
    in_maps = prepare_inputs(obs, x0, A_T, Bw_T, By_T, Cv_T, Dvw_T, Dvy_T,
                             Cu_T, Duw_T, Duy_T)
    trace = bool(int(os.environ.get("RINN_TRACE", "0")))
    res = run_bass_kernel_spmd(nc, in_maps, core_ids=list(range(N_CORES)),
                               trace=trace)
    if trace:
        _CACHE["last_results"] = res
    return assemble_output(res.results, log_stds, t_pad)
